# revision 101
# baseline (speedup 1.0000x reference)
"""Multi-head causal attention (B=2, S=2048, E=1024, H=16, Dh=64) on 8 TRN2
NeuronCores.

Sharding: core c handles batch c//4 and the 4 heads [4*(c%4), 4*(c%4)+4).
Each core computes its heads' QKV projections, causal softmax attention, and
a partial output projection (contraction over its 256 d_inner columns).
The host sums the 4 partial outputs per batch (the "all-reduce") and adds
bo' = bo + bv @ Wo (the V bias commutes through attention since softmax rows
sum to 1; the K bias shifts every logit in a row equally so it drops out).

Device layout notes:
  - Activations and QKV weights are shipped as bf16 (halves the input DMA
    stream); all PSUM accumulation is fp32 and everything downstream of the
    projections is fp32/f32r.
  - Q,K are produced transposed (d-major [d, s]); V is seq-major [s, d];
    attention scores are computed transposed [k, q] so the softmax sum is a
    matmul-reduction over partitions (ones column rides in the V tile).
  - Attention runs in causal q chunks with trimmed diagonal matmuls; the
    output projection and the second half of the V projection are spread
    through the score/PV loop as PE filler work so the exp throughput on
    the Activation engine never stalls the Tensor engine.
"""

import numpy as np
import ml_dtypes

import concourse.bass as bass
import concourse.tile as tile
from concourse import bacc, mybir
from concourse.bass_utils import run_bass_kernel_spmd

F32 = mybir.dt.float32
F32R = mybir.dt.float32r
BF16 = mybir.dt.bfloat16

B, S, E = 2, 2048, 1024
H, DH = 16, 64
NCORES = 8
HPC = 4          # heads per core
DL = HPC * DH    # 256: d_inner slice per core
NKT = E // 128   # 8  k-tiles over embed dim
NST = S // 128   # 16 seq tiles of 128

ExpF = mybir.ActivationFunctionType.Exp
IdF = mybir.ActivationFunctionType.Identity


def build_nc():
    nc = bacc.Bacc("TRN2", target_bir_lowering=False)

    xt_d = nc.dram_tensor("xt", [E, S], BF16, kind="ExternalInput")
    wq_d = nc.dram_tensor("wq", [E, DL], BF16, kind="ExternalInput")
    wk_d = nc.dram_tensor("wk", [E, DL], BF16, kind="ExternalInput")
    wv_d = nc.dram_tensor("wv", [E, DL], BF16, kind="ExternalInput")
    wo_d = nc.dram_tensor("wo", [DL, E], F32R, kind="ExternalInput")
    bqc_d = nc.dram_tensor("bqc", [DL, 1], F32, kind="ExternalInput")
    vone_d = nc.dram_tensor("v1ones", [128, 2 * HPC], BF16,
                            kind="ExternalInput")
    mask_d = nc.dram_tensor("masks", [128, 4, 1024], BF16,
                            kind="ExternalInput")
    out_d = nc.dram_tensor("out", [E, S], BF16, kind="ExternalOutput")

    with tile.TileContext(nc) as tc:
        with (
            tc.tile_pool(name="const", bufs=1) as cp,
            tc.tile_pool(name="ptp", bufs=28) as ptp,
        ):
            bqc = [cp.tile([128, 1], F32, tag=f"bqc{m}", name=f"bqc{m}")
                   for m in range(2)]

            qt = [cp.tile([128, S], F32R, tag=f"qt{m}", name=f"qt{m}")
                  for m in range(2)]
            kt = [cp.tile([128, S], F32R, tag=f"kt{m}", name=f"kt{m}")
                  for m in range(2)]
            ot = [cp.tile([128, S], F32R, tag=f"ot{m}", name=f"ot{m}")
                  for m in range(2)]
            v1 = [cp.tile([128, 2 * HPC * 65], BF16, tag=f"v1{s}",
                          name=f"v1{s}") for s in range(NST // 2)]
            wvt = cp.tile([128, NKT, DL], BF16, tag="wvt")
            wv = [wvt[:, k, :] for k in range(NKT)]
            wo = [cp.tile([128, E], F32R, tag=f"wo{d}", name=f"wo{d}")
                  for d in range(2)]
            maskt = cp.tile([128, 4, 1024], BF16, tag="maskt")
            masks = [maskt[:, j, :] for j in range(4)]
            wtqf = cp.tile([128, NKT, DL], BF16, tag="wtqf")
            wtkf = cp.tile([128, NKT, DL], BF16, tag="wtkf")
            wtq = [wtqf[:, :, m * 128:(m + 1) * 128] for m in range(2)]
            wtk = [wtkf[:, :, m * 128:(m + 1) * 128] for m in range(2)]
            # xt low columns (seq 0..1024) are only needed in phase 1; the
            # high half feeds the deferred V projection inside attention,
            # so it lives in the persistent pool.
            xtB = [cp.tile([128, S // 2], BF16, tag=f"xtB{k}",
                           name=f"xtB{k}") for k in range(NKT)]

            # pt (exp'd probability) tiles span both phases: early attention
            # blocks are score-computed inside phase 1 while the Activation
            # engine is otherwise idle.
            pend_map = {}   # (q0, hp) -> [(kb, pt, first, last, off), ...]

            def scores_kb(q0, qw, hp, i, kb, st_alloc):
                """One k block of scores for chunk block (q0, hp): matmul
                into a fresh stile, exp (+causal mask) into a pt tile, and
                append to the block's pending list."""
                nkb = (q0 + qw) // 128
                kb0 = q0 // 128
                j = kb - kb0
                off = (0 if qw != 512
                       else {1: 128, 2: 256, 3: 256}.get(j, 0))
                stile = st_alloc()
                ksl = slice(kb * 128, kb * 128 + 128)
                for h in range(2):
                    nc.tensor.matmul(
                        stile[:, h * qw + off:(h + 1) * qw],
                        kt[hp][h * 64:h * 64 + 64, ksl],
                        qt[hp][h * 64:h * 64 + 64, q0 + off:q0 + qw],
                        start=True, stop=True,
                    )
                pt = ptp.tile([128, 1024], BF16, tag="pt", name="pt")
                if off:
                    for h in range(2):
                        hs = slice(h * qw + off, (h + 1) * qw)
                        nc.scalar.activation(pt[:, hs], stile[:, hs],
                                             ExpF, scale=0.125)
                    with nc.allow_low_precision(
                            reason="0/1 mask multiply"):
                        for h in range(2):
                            hs = slice(h * qw + off, (h + 1) * qw)
                            nc.vector.tensor_mul(pt[:, hs], pt[:, hs],
                                                 masks[j][:, hs])
                else:
                    nc.scalar.activation(pt[:, 0:2 * qw], stile[:, 0:2 * qw],
                                         ExpF, scale=0.125)
                    if j >= 0:
                        with nc.allow_low_precision(
                                reason="0/1 mask multiply"):
                            nc.vector.tensor_mul(pt[:, 0:2 * qw],
                                                 pt[:, 0:2 * qw],
                                                 masks[j][:, 0:2 * qw])
                pend_map.setdefault((q0, hp), []).append(
                    (kb, pt, i == 0, i == nkb - 1, off))

            def diag_first(q0, qw):
                nkb = (q0 + qw) // 128
                kb0 = q0 // 128
                return list(range(kb0, nkb)) + list(range(0, kb0))

            # ============ phase 1 ============
            with (
                tc.tile_pool(name="xtp", bufs=1) as xp,
                tc.tile_pool(name="psqk", bufs=2, space="PSUM") as pqk,
            ):
                xtA = [xp.tile([128, S // 2], BF16, tag=f"xtA{k}",
                               name=f"xtA{k}") for k in range(NKT)]

                def xts(k, sb):
                    t = xtA[k] if sb < 2 else xtB[k]
                    return t[:, (sb % 2) * 512:(sb % 2) * 512 + 512]

                # DMA queue order = need order: first xt half-tile, k=0
                # weight slivers, rest of m=0 weights, small consts, the xt
                # stream, then V/m=1 weights, then attention-phase consts.
                nc.sync.dma_start(out=wtqf[:, 0, :], in_=wq_d[0:128, :])
                nc.sync.dma_start(out=xtA[0][:, 0:512],
                                  in_=xt_d[0:128, 0:512])
                nc.sync.dma_start(out=wtkf[:, 0, :], in_=wk_d[0:128, :])
                nc.sync.dma_start(out=xtA[0][:, 512:1024],
                                  in_=xt_d[0:128, 512:1024])
                nc.sync.dma_start(out=xtB[0][:], in_=xt_d[0:128, 1024:2048])
                nc.sync.dma_start(out=xtA[1][:],
                                  in_=xt_d[128:256, 0:1024])
                nc.sync.dma_start(
                    out=wtqf[:, 1:, :],
                    in_=wq_d[128:, :].rearrange("(k p) c -> p k c", p=128))
                nc.sync.dma_start(
                    out=wtkf[:, 1:, :],
                    in_=wk_d[128:, :].rearrange("(k p) c -> p k c", p=128))
                nc.sync.dma_start(out=xtB[1][:], in_=xt_d[128:256,
                                                          1024:2048])
                for mm in range(2):
                    nc.sync.dma_start(out=bqc[mm][:],
                                      in_=bqc_d[mm * 128:(mm + 1) * 128, :])
                for k in range(2, NKT):
                    nc.sync.dma_start(out=xtA[k][:],
                                      in_=xt_d[k * 128:(k + 1) * 128,
                                               0:1024])
                    nc.sync.dma_start(out=xtB[k][:],
                                      in_=xt_d[k * 128:(k + 1) * 128,
                                               1024:2048])
                nc.sync.dma_start(
                    out=wvt[:, 0:4, :],
                    in_=wv_d[0:512, :].rearrange("(k p) c -> p k c", p=128))
                nc.sync.dma_start(
                    out=wvt[:, 4:, :],
                    in_=wv_d[512:, :].rearrange("(k p) c -> p k c", p=128))
                nc.sync.dma_start(out=maskt[:], in_=mask_d[:])
                for s in range(NST // 2):
                    nc.sync.dma_start(
                        out=v1[s].rearrange("p (h c) -> p h c",
                                            c=65)[:, :, 64:65],
                        in_=vone_d.rearrange("p (h c) -> p h c", c=1)[:],
                    )
                for d in range(2):
                    nc.sync.dma_start(out=wo[d][:],
                                      in_=wo_d[d * 128:(d + 1) * 128, :])

                # -- Q and K projections, interleaved under the xt stream --
                # qt[m][d, s] = ((X @ Wq + bq).T)[m*128:(m+1)*128, :]
                def qk_proj(m):
                    pq = [pqk.tile([128, 1024], F32, tag="pq", name="pq")
                          for _ in range(2)]
                    pk = [pqk.tile([128, 1024], F32, tag="pk", name="pk")
                          for _ in range(2)]
                    for k in range(NKT):
                        for sb in range(4):
                            for (wt, ps) in ((wtq[m], pq), (wtk[m], pk)):
                                nc.tensor.matmul(
                                    ps[sb // 2][:,
                                                (sb % 2) * 512:
                                                (sb % 2) * 512 + 512],
                                    wt[:, k, :],
                                    xts(k, sb),
                                    start=(k == 0), stop=(k == NKT - 1),
                                )
                    # attention starts diagonal-first on the q block at
                    # 512:1024, so those kt/qt columns go out first (on
                    # separate engines); the ps[1] readers can be deferred
                    # so they don't delay releasing the ps[0] PSUM banks
                    nc.vector.tensor_copy(kt[m][:, 512:1024],
                                          pk[0][:, 512:1024])
                    nc.scalar.activation(qt[m][:, 512:1024],
                                         pq[0][:, 512:1024],
                                         IdF, bias=bqc[m][:])
                    nc.vector.tensor_copy(kt[m][:, 0:512], pk[0][:, 0:512])
                    nc.scalar.activation(qt[m][:, 0:512], pq[0][:, 0:512],
                                         IdF, bias=bqc[m][:])

                    def ps1_epilogue():
                        with nc.allow_low_precision(
                                reason="f32r round of q + bias"):
                            nc.vector.tensor_scalar_add(
                                qt[m][:, 1024:2048], pq[1][:], bqc[m][:])
                        nc.scalar.activation(kt[m][:, 1024:2048],
                                             pk[1][:], IdF)
                    return ps1_epilogue

                qk0_ps1 = qk_proj(0)

                # -- V projection, first half (activations stationary) --
                # v1[st][s, 65h:65h+64] = (X @ Wv)[st*128.., 64h:64h+64];
                # second half (st 8..15) is emitted later as PE filler work
                # inside the attention phase, since causal PV only touches
                # high k blocks late.  The hp=0 score+exp work of the first
                # two attention chunks rides along here: the Activation
                # engine is idle in phase 1, and pulling those exps forward
                # keeps it off the critical path later.
                st_eng = [0]

                def st_alloc():
                    st_eng[0] ^= 1
                    return pqk.tile([128, 1024], F32,
                                    tag=("pq" if st_eng[0] else "pk"),
                                    name="sa_stile")

                ahead = [(512, 512, 0, i, kb)
                         for i, kb in enumerate(diag_first(512, 512))]
                ahead += [(0, 512, 0, i, kb)
                          for i, kb in enumerate(diag_first(0, 512))]
                ahead += [(1024, 512, 0, i, kb)
                          for i, kb in enumerate(diag_first(1024, 512))]
                for sp2 in range(NST // 4):
                    pv = pqk.tile([128, 512], F32,
                                  tag=("pq" if sp2 % 2 else "pk"), name="pv")
                    for i in range(2):
                        st = 2 * sp2 + i
                        for k in range(NKT):
                            nc.tensor.matmul(
                                pv[:, i * DL:(i + 1) * DL],
                                xtA[k][:, st * 128:(st + 1) * 128],
                                wv[k],
                                start=(k == 0), stop=(k == NKT - 1),
                            )
                    vv = v1[sp2].rearrange("p (h c) -> p h c", c=65)
                    pvv = pv[:].rearrange("p (h c) -> p h c", c=64)
                    if sp2 % 2:
                        nc.scalar.copy(out=vv[:, :, 0:64], in_=pvv[:])
                    else:
                        nc.vector.tensor_copy(vv[:, :, 0:64], pvv[:])
                    for _ in range(3):
                        if ahead:
                            scores_kb(*ahead.pop(0), st_alloc)
                    if sp2 == 0:
                        qk0_ps1()
                while ahead:
                    scores_kb(*ahead.pop(0), st_alloc)

                qk_proj(1)()

            # ============ phase 2 (xtA region reused) ============
            with (
                tc.tile_pool(name="small", bufs=2) as sp,
                tc.tile_pool(name="oev", bufs=2) as op,
                tc.tile_pool(name="psatt", bufs=1, space="PSUM") as pat,
            ):
                fillers = []

                def pop_filler(reserve=0):
                    if len(fillers) > reserve:
                        fillers.pop(0)()

                copy_eng = [0]

                # one output-staging buffer per chunk: 8 et copies land
                # here, then a single batched DMA writes the whole chunk
                oeb = {}

                def oproj(q0, qw, et):
                    p3 = pat.tile([128, 512], F32, tag="pp", bufs=2,
                                  name="p3")
                    esl = slice(et * 128, et * 128 + 128)
                    for d in range(2):
                        nc.tensor.matmul(
                            p3[:, 0:qw],
                            wo[d][:, esl],
                            ot[d][:, q0:q0 + qw],
                            start=(d == 0), stop=(d == 1),
                        )
                    if q0 not in oeb:
                        oeb[q0] = op.tile([128, NKT, 512], BF16, tag="oe",
                                          name="oe")
                    oe = oeb[q0]
                    copy_eng[0] ^= 1
                    if copy_eng[0]:
                        nc.scalar.copy(out=oe[:, et, 0:qw], in_=p3[:, 0:qw])
                    else:
                        nc.vector.tensor_copy(oe[:, et, 0:qw], p3[:, 0:qw])
                    if et % 2 == 1:
                        nc.sync.dma_start(
                            out=out_d[(et - 1) * 128:(et + 1) * 128,
                                      q0:q0 + qw].rearrange(
                                "(e p) q -> p e q", p=128),
                            in_=oe[:, et - 1:et + 1, 0:qw],
                        )
                        if et == NKT - 1:
                            del oeb[q0]

                def vproj_st(st):
                    # one 128-seq tile of the deferred V projection
                    pv = pat.tile([128, 512], F32, tag="pp", bufs=2,
                                  name="pv")
                    for k in range(NKT):
                        nc.tensor.matmul(
                            pv[:, 0:DL],
                            xtB[k][:, (st - 8) * 128:(st - 7) * 128],
                            wv[k],
                            start=(k == 0), stop=(k == NKT - 1),
                        )
                    g = (st % 2) * 260
                    vv = v1[st // 2][:, g:g + 260].rearrange(
                        "p (h c) -> p h c", c=65)
                    pvv = pv[:, 0:DL].rearrange("p (h c) -> p h c", c=64)
                    nc.vector.tensor_copy(vv[:, :, 0:64], pvv[:])

                # ---- globally pipelined attention ----
                # Blocks run one behind: block n's PV drain and softmax
                # normalize interleave through block n+1's score stream, so
                # reciprocal/broadcast latency hides behind stiles and the
                # Act-vs-PE rate gap is bridged by oproj/V filler pops.
                chunks = [(512, 512), (0, 512), (1024, 512), (1536, 512)]
                fillers += [(lambda st=st: vproj_st(st))
                            for st in range(8, 16)]

                def st_alloc():
                    return pat.tile([128, 1024], F32, tag="st",
                                    bufs=2, name="stile")

                po_of = {}    # block idx -> (po_a, po_b)
                live = []     # FIFO of (block idx, pend entry)
                blocks = [dict(q0=q0, qw=qw, hp=hp)
                          for (q0, qw) in chunks for hp in range(2)]

                def normalize(b):
                    # recip denominators land on partition 0 (vector ops may
                    # shift partition offsets), partition_broadcast fans them
                    # out (it can only read partition 0), and the multiplies
                    # write both ot halves in place (shifting up for h=1).
                    q0, qw, hp = b["q0"], b["qw"], b["hp"]
                    po_a, po_b = po_of.pop(b["idx"])
                    rb = sp.tile([1, 1024], F32R, tag="rb", name="rb")
                    with nc.allow_low_precision(
                            reason="f32r rounding of softmax denoms"):
                        nc.vector.reciprocal(rb[0:1, 0:qw],
                                             po_b[64:65, 0:qw])
                        nc.vector.reciprocal(rb[0:1, 512:512 + qw],
                                             po_a[64:65, 0:qw])
                    for h, po in ((1, po_b), (0, po_a)):
                        c0 = 0 if h else 512
                        rbb = sp.tile([64, 512], F32R, tag="rbb", name="rbb")
                        nc.gpsimd.partition_broadcast(
                            rbb[:, 0:qw], rb[0:1, c0:c0 + qw])
                        nc.vector.tensor_mul(
                            ot[hp][h * 64:h * 64 + 64, q0:q0 + qw],
                            po[0:64, 0:qw], rbb[:, 0:qw])
                    if hp == 1:
                        fillers.extend(
                            [(lambda q0=q0, qw=qw, et=et: oproj(q0, qw, et))
                             for et in range(NKT)])

                def pv_pop():
                    bi, pend = live.pop(0)
                    b = blocks[bi]
                    qw, hp = b["qw"], b["hp"]
                    if bi not in po_of:
                        po_of[bi] = (
                            pat.tile([128, 512], F32, tag="po", bufs=2,
                                     name="po_a"),
                            pat.tile([128, 512], F32, tag="po", bufs=2,
                                     name="po_b"))
                    po_a, po_b = po_of[bi]
                    pkb, ppt, pfirst, plast, poff = pend
                    base = (pkb % 2) * 260
                    for h, po in ((0, po_a), (1, po_b)):
                        lh = 2 * hp + h
                        nc.tensor.matmul(
                            po[0:65, poff:qw],
                            v1[pkb // 2][:, base + lh * 65:
                                         base + lh * 65 + 65],
                            ppt[:, h * qw + poff:(h + 1) * qw],
                            start=pfirst, stop=plast,
                            skip_group_check=True,
                        )
                    if plast:
                        normalize(b)

                nsc = [0]
                krsv = [4]

                def after_score():
                    nsc[0] += 1
                    while len(live) >= 12:
                        pv_pop()
                    if nsc[0] % 3 == 0:
                        pop_filler(krsv[0])

                for bi, b in enumerate(blocks):
                    b["idx"] = bi
                    # hold fillers back mid-schedule; spend them at the
                    # final drain where the score stream has ended
                    krsv[0] = 6 if bi == len(blocks) - 1 else (
                        0 if b["q0"] == 0 else (4 if nsc[0] <= 40 else 0))
                    q0, qw, hp = b["q0"], b["qw"], b["hp"]
                    pre = pend_map.pop((q0, hp), [])
                    npre = len(pre)
                    live.extend((bi, p) for p in pre)
                    nkb = (q0 + qw) // 128
                    if npre >= nkb:
                        # fully preloaded: drain burst keeps PE fed while
                        # the Activation engine idles here
                        while len(live) >= 12:
                            pv_pop()
                        continue
                    # drain anything two blocks back before a new po pair
                    # could be needed mid-stream
                    while live and live[0][0] <= bi - 2:
                        pv_pop()
                    for i, kb in enumerate(diag_first(q0, qw)):
                        if i < npre:
                            continue
                        scores_kb(q0, qw, hp, i, kb, st_alloc)
                        live.append((bi, pend_map[(q0, hp)].pop(0)))
                        after_score()
                    pend_map.pop((q0, hp), None)
                npv = [0]
                while live:
                    pv_pop()
                    npv[0] += 1
                    if npv[0] % 2 == 0:
                        pop_filler()
                while fillers:
                    pop_filler()

    nc.compile()
    return nc


def _make_masks(qw, nj):
    kk = np.arange(128)[:, None]
    qq = np.arange(qw)[None, :]
    ms = []
    for j in range(nj):
        m = np.where(qq >= kk + 128 * j, 1.0, 0.0).astype(np.float32)
        ms.append(np.concatenate([m, m], axis=1))
    return np.ascontiguousarray(np.stack(ms).transpose(1, 0, 2))


_NC = None


def _get_nc():
    global _NC
    if _NC is None:
        _NC = build_nc()
    return _NC


def make_in_maps(inputs, Wq, bq, Wk, Wv, Wo):
    bf = ml_dtypes.bfloat16
    masks = _make_masks(512, 4).astype(bf)
    vones = np.ones((128, 2 * HPC), bf)
    in_maps = []
    for c in range(NCORES):
        b, g = c // HPC, c % HPC
        sl = slice(g * DL, (g + 1) * DL)
        in_maps.append({
            "xt": np.ascontiguousarray(np.asarray(inputs[b]).T.astype(bf)),
            "wq": np.ascontiguousarray(Wq[:, sl].astype(bf)),
            "wk": np.ascontiguousarray(Wk[:, sl].astype(bf)),
            "wv": np.ascontiguousarray(Wv[:, sl].astype(bf)),
            "wo": np.ascontiguousarray(Wo[sl, :]),
            "bqc": bq[sl].reshape(DL, 1),
            "v1ones": vones,
            "masks": masks,
        })
    return in_maps


def assemble(results):
    outs = []
    for b in range(B):
        acc = results[b * HPC]["out"].astype(np.float32).copy()
        for g in range(1, HPC):
            acc += results[b * HPC + g]["out"]
        outs.append(acc.T)
    return np.stack(outs)


def kernel(inputs, Wq, bq, Wk, bk, Wv, bv, Wo, bo):
    inputs = np.asarray(inputs, np.float32)
    Wq, bq, Wk, Wv, Wo = (np.asarray(a, np.float32)
                          for a in (Wq, bq, Wk, Wv, Wo))
    bv = np.asarray(bv, np.float32)
    bo_eff = np.asarray(bo, np.float32) + bv @ np.asarray(Wo, np.float32)
    in_maps = make_in_maps(inputs, Wq, bq, Wk, Wv, Wo)
    nc = _get_nc()
    res = run_bass_kernel_spmd(nc, in_maps, list(range(NCORES)))
    out = assemble(res.results)
    return (out + bo_eff).astype(np.float32)


# revision 102
# speedup vs baseline: 1.0409x; 1.0409x over previous
"""Multi-head causal attention (B=2, S=2048, E=1024, H=16, Dh=64) on 8 TRN2
NeuronCores.

Sharding: core c handles batch c//4 and the 4 heads [4*(c%4), 4*(c%4)+4).
Each core computes its heads' QKV projections, causal softmax attention, and
a partial output projection (contraction over its 256 d_inner columns).
The host sums the 4 partial outputs per batch (the "all-reduce") and adds
bo' = bo + bv @ Wo (the V bias commutes through attention since softmax rows
sum to 1; the K bias shifts every logit in a row equally so it drops out).

Device layout notes:
  - Activations and QKV weights are shipped as bf16 (halves the input DMA
    stream); all PSUM accumulation is fp32 and everything downstream of the
    projections is fp32/f32r.
  - Q,K are produced transposed (d-major [d, s]); V is seq-major [s, d];
    attention scores are computed transposed [k, q] so the softmax sum is a
    matmul-reduction over partitions (ones column rides in the V tile).
  - Attention runs in causal q chunks with trimmed diagonal matmuls; the
    output projection and the second half of the V projection are spread
    through the score/PV loop as PE filler work so the exp throughput on
    the Activation engine never stalls the Tensor engine.
"""

import numpy as np
import ml_dtypes

import concourse.bass as bass
import concourse.tile as tile
from concourse import bacc, mybir
from concourse.bass_utils import run_bass_kernel_spmd

F32 = mybir.dt.float32
F32R = mybir.dt.float32r
BF16 = mybir.dt.bfloat16

B, S, E = 2, 2048, 1024
H, DH = 16, 64
NCORES = 8
HPC = 4          # heads per core
DL = HPC * DH    # 256: d_inner slice per core
NKT = E // 128   # 8  k-tiles over embed dim
NST = S // 128   # 16 seq tiles of 128

ExpF = mybir.ActivationFunctionType.Exp
IdF = mybir.ActivationFunctionType.Identity


def build_nc():
    nc = bacc.Bacc("TRN2", target_bir_lowering=False)

    xt_d = nc.dram_tensor("xt", [E, S], BF16, kind="ExternalInput")
    wq_d = nc.dram_tensor("wq", [E, DL], BF16, kind="ExternalInput")
    wk_d = nc.dram_tensor("wk", [E, DL], BF16, kind="ExternalInput")
    wv_d = nc.dram_tensor("wv", [E, DL], BF16, kind="ExternalInput")
    wo_d = nc.dram_tensor("wo", [DL, E], F32R, kind="ExternalInput")
    bqc_d = nc.dram_tensor("bqc", [DL, 1], F32, kind="ExternalInput")
    vone_d = nc.dram_tensor("v1ones", [128, 2 * HPC], BF16,
                            kind="ExternalInput")
    mask_d = nc.dram_tensor("masks", [128, 4, 1024], BF16,
                            kind="ExternalInput")
    out_d = nc.dram_tensor("out", [E, S], BF16, kind="ExternalOutput")

    with tile.TileContext(nc) as tc:
        with (
            tc.tile_pool(name="const", bufs=1) as cp,
            tc.tile_pool(name="ptp", bufs=16) as ptp,
        ):
            bqc = [cp.tile([128, 1], F32, tag=f"bqc{m}", name=f"bqc{m}")
                   for m in range(2)]

            qt = [cp.tile([128, S], F32R, tag=f"qt{m}", name=f"qt{m}")
                  for m in range(2)]
            kt = [cp.tile([128, S], F32R, tag=f"kt{m}", name=f"kt{m}")
                  for m in range(2)]
            ot = [cp.tile([128, S], F32R, tag=f"ot{m}", name=f"ot{m}")
                  for m in range(2)]
            v1 = [cp.tile([128, 2 * HPC * 65], BF16, tag=f"v1{s}",
                          name=f"v1{s}") for s in range(NST // 2)]
            wvt = cp.tile([128, NKT, DL], BF16, tag="wvt")
            wv = [wvt[:, k, :] for k in range(NKT)]
            wo = [cp.tile([128, E], F32R, tag=f"wo{d}", name=f"wo{d}")
                  for d in range(2)]
            maskt = cp.tile([128, 4, 1024], BF16, tag="maskt")
            masks = [maskt[:, j, :] for j in range(4)]
            wtqf = cp.tile([128, NKT, DL], BF16, tag="wtqf")
            wtkf = cp.tile([128, NKT, DL], BF16, tag="wtkf")
            wtq = [wtqf[:, :, m * 128:(m + 1) * 128] for m in range(2)]
            wtk = [wtkf[:, :, m * 128:(m + 1) * 128] for m in range(2)]
            # xt low columns (seq 0..1024) are only needed in phase 1; the
            # high half feeds the deferred V projection inside attention,
            # so it lives in the persistent pool.
            xtB = [cp.tile([128, S // 2], BF16, tag=f"xtB{k}",
                           name=f"xtB{k}") for k in range(NKT)]

            # pt (exp'd probability) tiles span both phases: early attention
            # blocks are score-computed inside phase 1 while the Activation
            # engine is otherwise idle.
            pend_map = {}   # (q0, hp) -> [(kb, pt, first, last, off), ...]

            def scores_kb(q0, qw, hp, i, kb, st_alloc):
                """One k block of scores for chunk block (q0, hp): matmul
                into a fresh stile, exp (+causal mask) into a pt tile, and
                append to the block's pending list."""
                nkb = (q0 + qw) // 128
                kb0 = q0 // 128
                j = kb - kb0
                off = (0 if qw != 512
                       else {1: 128, 2: 256, 3: 256}.get(j, 0))
                stile = st_alloc()
                ksl = slice(kb * 128, kb * 128 + 128)
                for h in range(2):
                    nc.tensor.matmul(
                        stile[:, h * qw + off:(h + 1) * qw],
                        kt[hp][h * 64:h * 64 + 64, ksl],
                        qt[hp][h * 64:h * 64 + 64, q0 + off:q0 + qw],
                        start=True, stop=True,
                    )
                pt = ptp.tile([128, 1024], BF16, tag="pt", name="pt")
                if off:
                    for h in range(2):
                        hs = slice(h * qw + off, (h + 1) * qw)
                        nc.scalar.activation(pt[:, hs], stile[:, hs],
                                             ExpF, scale=0.125)
                    with nc.allow_low_precision(
                            reason="0/1 mask multiply"):
                        for h in range(2):
                            hs = slice(h * qw + off, (h + 1) * qw)
                            nc.vector.tensor_mul(pt[:, hs], pt[:, hs],
                                                 masks[j][:, hs])
                else:
                    nc.scalar.activation(pt[:, 0:2 * qw], stile[:, 0:2 * qw],
                                         ExpF, scale=0.125)
                    if j >= 0:
                        with nc.allow_low_precision(
                                reason="0/1 mask multiply"):
                            nc.vector.tensor_mul(pt[:, 0:2 * qw],
                                                 pt[:, 0:2 * qw],
                                                 masks[j][:, 0:2 * qw])
                pend_map.setdefault((q0, hp), []).append(
                    (kb, pt, i == 0, i == nkb - 1, off))

            def diag_first(q0, qw):
                nkb = (q0 + qw) // 128
                kb0 = q0 // 128
                return list(range(kb0, nkb)) + list(range(0, kb0))

            # ============ phase 1 ============
            with (
                tc.tile_pool(name="xtp", bufs=1) as xp,
                tc.tile_pool(name="psqk", bufs=2, space="PSUM") as pqk,
            ):
                xtA = [xp.tile([128, S // 2], BF16, tag=f"xtA{k}",
                               name=f"xtA{k}") for k in range(NKT)]

                def xts(k, sb):
                    t = xtA[k] if sb < 2 else xtB[k]
                    return t[:, (sb % 2) * 512:(sb % 2) * 512 + 512]

                # DMA queue order = need order: first xt half-tile, k=0
                # weight slivers, rest of m=0 weights, small consts, the xt
                # stream, then V/m=1 weights, then attention-phase consts.
                nc.sync.dma_start(out=wtqf[:, 0, :], in_=wq_d[0:128, :])
                nc.sync.dma_start(out=xtA[0][:, 0:512],
                                  in_=xt_d[0:128, 0:512])
                nc.sync.dma_start(out=wtkf[:, 0, :], in_=wk_d[0:128, :])
                nc.sync.dma_start(out=xtA[0][:, 512:1024],
                                  in_=xt_d[0:128, 512:1024])
                nc.sync.dma_start(out=xtB[0][:], in_=xt_d[0:128, 1024:2048])
                nc.sync.dma_start(out=xtA[1][:],
                                  in_=xt_d[128:256, 0:1024])
                nc.sync.dma_start(
                    out=wtqf[:, 1:, :],
                    in_=wq_d[128:, :].rearrange("(k p) c -> p k c", p=128))
                nc.sync.dma_start(
                    out=wtkf[:, 1:, :],
                    in_=wk_d[128:, :].rearrange("(k p) c -> p k c", p=128))
                nc.sync.dma_start(out=xtB[1][:], in_=xt_d[128:256,
                                                          1024:2048])
                for mm in range(2):
                    nc.sync.dma_start(out=bqc[mm][:],
                                      in_=bqc_d[mm * 128:(mm + 1) * 128, :])
                for k in range(2, NKT):
                    nc.sync.dma_start(out=xtA[k][:],
                                      in_=xt_d[k * 128:(k + 1) * 128,
                                               0:1024])
                    nc.sync.dma_start(out=xtB[k][:],
                                      in_=xt_d[k * 128:(k + 1) * 128,
                                               1024:2048])
                nc.sync.dma_start(
                    out=wvt[:, 0:4, :],
                    in_=wv_d[0:512, :].rearrange("(k p) c -> p k c", p=128))
                nc.sync.dma_start(
                    out=wvt[:, 4:, :],
                    in_=wv_d[512:, :].rearrange("(k p) c -> p k c", p=128))
                nc.sync.dma_start(out=maskt[:], in_=mask_d[:])
                for s in range(NST // 2):
                    nc.sync.dma_start(
                        out=v1[s].rearrange("p (h c) -> p h c",
                                            c=65)[:, :, 64:65],
                        in_=vone_d.rearrange("p (h c) -> p h c", c=1)[:],
                    )
                for d in range(2):
                    nc.sync.dma_start(out=wo[d][:],
                                      in_=wo_d[d * 128:(d + 1) * 128, :])

                # -- Q and K projections, interleaved under the xt stream --
                # qt[m][d, s] = ((X @ Wq + bq).T)[m*128:(m+1)*128, :]
                def qk_proj(m):
                    pq = [pqk.tile([128, 1024], F32, tag="pq", name="pq")
                          for _ in range(2)]
                    pk = [pqk.tile([128, 1024], F32, tag="pk", name="pk")
                          for _ in range(2)]
                    for k in range(NKT):
                        for sb in range(4):
                            for (wt, ps) in ((wtq[m], pq), (wtk[m], pk)):
                                nc.tensor.matmul(
                                    ps[sb // 2][:,
                                                (sb % 2) * 512:
                                                (sb % 2) * 512 + 512],
                                    wt[:, k, :],
                                    xts(k, sb),
                                    start=(k == 0), stop=(k == NKT - 1),
                                )
                    # attention starts diagonal-first on the q block at
                    # 512:1024, so those kt/qt columns go out first (on
                    # separate engines); the ps[1] readers can be deferred
                    # so they don't delay releasing the ps[0] PSUM banks
                    nc.vector.tensor_copy(kt[m][:, 512:1024],
                                          pk[0][:, 512:1024])
                    nc.scalar.activation(qt[m][:, 512:1024],
                                         pq[0][:, 512:1024],
                                         IdF, bias=bqc[m][:])
                    nc.vector.tensor_copy(kt[m][:, 0:512], pk[0][:, 0:512])
                    nc.scalar.activation(qt[m][:, 0:512], pq[0][:, 0:512],
                                         IdF, bias=bqc[m][:])

                    def ps1_epilogue():
                        with nc.allow_low_precision(
                                reason="f32r round of q + bias"):
                            nc.vector.tensor_scalar_add(
                                qt[m][:, 1024:2048], pq[1][:], bqc[m][:])
                        nc.scalar.activation(kt[m][:, 1024:2048],
                                             pk[1][:], IdF)
                    return ps1_epilogue

                qk0_ps1 = qk_proj(0)

                # -- V projection, first half (activations stationary) --
                # v1[st][s, 65h:65h+64] = (X @ Wv)[st*128.., 64h:64h+64];
                # second half (st 8..15) is emitted later as PE filler work
                # inside the attention phase, since causal PV only touches
                # high k blocks late.  The hp=0 score+exp work of the first
                # two attention chunks rides along here: the Activation
                # engine is idle in phase 1, and pulling those exps forward
                # keeps it off the critical path later.
                st_eng = [0]

                def st_alloc():
                    st_eng[0] ^= 1
                    return pqk.tile([128, 1024], F32,
                                    tag=("pq" if st_eng[0] else "pk"),
                                    name="sa_stile")

                ahead = [(512, 512, 0, i, kb)
                         for i, kb in enumerate(diag_first(512, 512))]
                ahead += [(0, 512, 0, i, kb)
                          for i, kb in enumerate(diag_first(0, 512))]
                for sp2 in range(NST // 4):
                    pv = pqk.tile([128, 512], F32,
                                  tag=("pq" if sp2 % 2 else "pk"), name="pv")
                    for i in range(2):
                        st = 2 * sp2 + i
                        for k in range(NKT):
                            nc.tensor.matmul(
                                pv[:, i * DL:(i + 1) * DL],
                                xtA[k][:, st * 128:(st + 1) * 128],
                                wv[k],
                                start=(k == 0), stop=(k == NKT - 1),
                            )
                    vv = v1[sp2].rearrange("p (h c) -> p h c", c=65)
                    pvv = pv[:].rearrange("p (h c) -> p h c", c=64)
                    if sp2 % 2:
                        nc.scalar.copy(out=vv[:, :, 0:64], in_=pvv[:])
                    else:
                        nc.vector.tensor_copy(vv[:, :, 0:64], pvv[:])
                    for _ in range(3):
                        if ahead:
                            scores_kb(*ahead.pop(0), st_alloc)
                    if sp2 == 0:
                        qk0_ps1()
                while ahead:
                    scores_kb(*ahead.pop(0), st_alloc)

                qk_proj(1)()

            # ============ phase 2 (xtA region reused) ============
            with (
                tc.tile_pool(name="small", bufs=2) as sp,
                tc.tile_pool(name="oev", bufs=2) as op,
                tc.tile_pool(name="psatt", bufs=1, space="PSUM") as pat,
            ):
                fillers = []

                def pop_filler(reserve=0):
                    if len(fillers) > reserve:
                        fillers.pop(0)()

                copy_eng = [0]

                # one output-staging buffer per chunk: 8 et copies land
                # here, then a single batched DMA writes the whole chunk
                oeb = {}

                def oproj(q0, qw, et):
                    p3 = pat.tile([128, 512], F32, tag="pp", bufs=2,
                                  name="p3")
                    esl = slice(et * 128, et * 128 + 128)
                    for d in range(2):
                        nc.tensor.matmul(
                            p3[:, 0:qw],
                            wo[d][:, esl],
                            ot[d][:, q0:q0 + qw],
                            start=(d == 0), stop=(d == 1),
                        )
                    if q0 not in oeb:
                        oeb[q0] = op.tile([128, NKT, 512], BF16, tag="oe",
                                          name="oe")
                    oe = oeb[q0]
                    copy_eng[0] ^= 1
                    if copy_eng[0]:
                        nc.scalar.copy(out=oe[:, et, 0:qw], in_=p3[:, 0:qw])
                    else:
                        nc.vector.tensor_copy(oe[:, et, 0:qw], p3[:, 0:qw])
                    if et % 2 == 1:
                        nc.sync.dma_start(
                            out=out_d[(et - 1) * 128:(et + 1) * 128,
                                      q0:q0 + qw].rearrange(
                                "(e p) q -> p e q", p=128),
                            in_=oe[:, et - 1:et + 1, 0:qw],
                        )
                        if et == NKT - 1:
                            del oeb[q0]

                def vproj_st(st):
                    # one 128-seq tile of the deferred V projection
                    pv = pat.tile([128, 512], F32, tag="pp", bufs=2,
                                  name="pv")
                    for k in range(NKT):
                        nc.tensor.matmul(
                            pv[:, 0:DL],
                            xtB[k][:, (st - 8) * 128:(st - 7) * 128],
                            wv[k],
                            start=(k == 0), stop=(k == NKT - 1),
                        )
                    g = (st % 2) * 260
                    vv = v1[st // 2][:, g:g + 260].rearrange(
                        "p (h c) -> p h c", c=65)
                    pvv = pv[:, 0:DL].rearrange("p (h c) -> p h c", c=64)
                    nc.vector.tensor_copy(vv[:, :, 0:64], pvv[:])

                # ---- globally pipelined attention ----
                # Blocks run one behind: block n's PV drain and softmax
                # normalize interleave through block n+1's score stream, so
                # reciprocal/broadcast latency hides behind stiles and the
                # Act-vs-PE rate gap is bridged by oproj/V filler pops.
                chunks = [(512, 512), (0, 512), (1024, 512), (1536, 512)]
                fillers += [(lambda st=st: vproj_st(st))
                            for st in range(8, 16)]

                def st_alloc():
                    return pat.tile([128, 1024], F32, tag="st",
                                    bufs=2, name="stile")

                po_of = {}    # block idx -> (po_a, po_b)
                live = []     # FIFO of (block idx, pend entry)
                blocks = [dict(q0=q0, qw=qw, hp=hp)
                          for (q0, qw) in chunks for hp in range(2)]

                def normalize(b):
                    # recip denominators land on partition 0 (vector ops may
                    # shift partition offsets), partition_broadcast fans them
                    # out (it can only read partition 0), and the multiplies
                    # write both ot halves in place (shifting up for h=1).
                    q0, qw, hp = b["q0"], b["qw"], b["hp"]
                    po_a, po_b = po_of.pop(b["idx"])
                    rb = sp.tile([1, 1024], F32R, tag="rb", name="rb")
                    with nc.allow_low_precision(
                            reason="f32r rounding of softmax denoms"):
                        nc.vector.reciprocal(rb[0:1, 0:qw],
                                             po_b[64:65, 0:qw])
                        nc.vector.reciprocal(rb[0:1, 512:512 + qw],
                                             po_a[64:65, 0:qw])
                    for h, po in ((1, po_b), (0, po_a)):
                        c0 = 0 if h else 512
                        rbb = sp.tile([64, 512], F32R, tag="rbb", name="rbb")
                        nc.gpsimd.partition_broadcast(
                            rbb[:, 0:qw], rb[0:1, c0:c0 + qw])
                        nc.vector.tensor_mul(
                            ot[hp][h * 64:h * 64 + 64, q0:q0 + qw],
                            po[0:64, 0:qw], rbb[:, 0:qw])
                    if hp == 1:
                        fillers.extend(
                            [(lambda q0=q0, qw=qw, et=et: oproj(q0, qw, et))
                             for et in range(NKT)])

                def pv_pop():
                    bi, pend = live.pop(0)
                    b = blocks[bi]
                    qw, hp = b["qw"], b["hp"]
                    if bi not in po_of:
                        po_of[bi] = (
                            pat.tile([128, 512], F32, tag="po", bufs=2,
                                     name="po_a"),
                            pat.tile([128, 512], F32, tag="po", bufs=2,
                                     name="po_b"))
                    po_a, po_b = po_of[bi]
                    pkb, ppt, pfirst, plast, poff = pend
                    base = (pkb % 2) * 260
                    for h, po in ((0, po_a), (1, po_b)):
                        lh = 2 * hp + h
                        nc.tensor.matmul(
                            po[0:65, poff:qw],
                            v1[pkb // 2][:, base + lh * 65:
                                         base + lh * 65 + 65],
                            ppt[:, h * qw + poff:(h + 1) * qw],
                            start=pfirst, stop=plast,
                            skip_group_check=True,
                        )
                    if plast:
                        normalize(b)

                nsc = [0]
                krsv = [4]

                def after_score():
                    nsc[0] += 1
                    while len(live) >= 12:
                        pv_pop()
                    if nsc[0] % 3 == 0:
                        pop_filler(krsv[0])

                for bi, b in enumerate(blocks):
                    b["idx"] = bi
                    # hold fillers back mid-schedule; spend them at the
                    # final drain where the score stream has ended
                    krsv[0] = 6 if bi == len(blocks) - 1 else (
                        0 if b["q0"] == 0 else (4 if nsc[0] <= 40 else 0))
                    q0, qw, hp = b["q0"], b["qw"], b["hp"]
                    pre = pend_map.pop((q0, hp), [])
                    npre = len(pre)
                    live.extend((bi, p) for p in pre)
                    nkb = (q0 + qw) // 128
                    if npre >= nkb:
                        # fully preloaded: drain burst keeps PE fed while
                        # the Activation engine idles here
                        while len(live) >= 12:
                            pv_pop()
                        continue
                    # drain anything two blocks back before a new po pair
                    # could be needed mid-stream
                    while live and live[0][0] <= bi - 2:
                        pv_pop()
                    for i, kb in enumerate(diag_first(q0, qw)):
                        if i < npre:
                            continue
                        scores_kb(q0, qw, hp, i, kb, st_alloc)
                        live.append((bi, pend_map[(q0, hp)].pop(0)))
                        after_score()
                    pend_map.pop((q0, hp), None)
                npv = [0]
                while live:
                    pv_pop()
                    npv[0] += 1
                    if npv[0] % 2 == 0:
                        pop_filler()
                while fillers:
                    pop_filler()

    nc.compile()
    return nc


def _make_masks(qw, nj):
    kk = np.arange(128)[:, None]
    qq = np.arange(qw)[None, :]
    ms = []
    for j in range(nj):
        m = np.where(qq >= kk + 128 * j, 1.0, 0.0).astype(np.float32)
        ms.append(np.concatenate([m, m], axis=1))
    return np.ascontiguousarray(np.stack(ms).transpose(1, 0, 2))


_NC = None


def _get_nc():
    global _NC
    if _NC is None:
        _NC = build_nc()
    return _NC


def make_in_maps(inputs, Wq, bq, Wk, Wv, Wo):
    bf = ml_dtypes.bfloat16
    masks = _make_masks(512, 4).astype(bf)
    vones = np.ones((128, 2 * HPC), bf)
    in_maps = []
    for c in range(NCORES):
        b, g = c // HPC, c % HPC
        sl = slice(g * DL, (g + 1) * DL)
        in_maps.append({
            "xt": np.ascontiguousarray(np.asarray(inputs[b]).T.astype(bf)),
            "wq": np.ascontiguousarray(Wq[:, sl].astype(bf)),
            "wk": np.ascontiguousarray(Wk[:, sl].astype(bf)),
            "wv": np.ascontiguousarray(Wv[:, sl].astype(bf)),
            "wo": np.ascontiguousarray(Wo[sl, :]),
            "bqc": bq[sl].reshape(DL, 1),
            "v1ones": vones,
            "masks": masks,
        })
    return in_maps


def assemble(results):
    outs = []
    for b in range(B):
        acc = results[b * HPC]["out"].astype(np.float32).copy()
        for g in range(1, HPC):
            acc += results[b * HPC + g]["out"]
        outs.append(acc.T)
    return np.stack(outs)


def kernel(inputs, Wq, bq, Wk, bk, Wv, bv, Wo, bo):
    inputs = np.asarray(inputs, np.float32)
    Wq, bq, Wk, Wv, Wo = (np.asarray(a, np.float32)
                          for a in (Wq, bq, Wk, Wv, Wo))
    bv = np.asarray(bv, np.float32)
    bo_eff = np.asarray(bo, np.float32) + bv @ np.asarray(Wo, np.float32)
    in_maps = make_in_maps(inputs, Wq, bq, Wk, Wv, Wo)
    nc = _get_nc()
    res = run_bass_kernel_spmd(nc, in_maps, list(range(NCORES)))
    out = assemble(res.results)
    return (out + bo_eff).astype(np.float32)


# revision 103
# speedup vs baseline: 1.0420x; 1.0010x over previous
"""Multi-head causal attention (B=2, S=2048, E=1024, H=16, Dh=64) on 8 TRN2
NeuronCores.

Sharding: core c handles batch c//4 and the 4 heads [4*(c%4), 4*(c%4)+4).
Each core computes its heads' QKV projections, causal softmax attention, and
a partial output projection (contraction over its 256 d_inner columns).
The host sums the 4 partial outputs per batch (the "all-reduce") and adds
bo' = bo + bv @ Wo (the V bias commutes through attention since softmax rows
sum to 1; the K bias shifts every logit in a row equally so it drops out).

Device layout notes:
  - Activations and QKV weights are shipped as bf16 (halves the input DMA
    stream); all PSUM accumulation is fp32 and everything downstream of the
    projections is fp32/f32r.
  - Q,K are produced transposed (d-major [d, s]); V is seq-major [s, d];
    attention scores are computed transposed [k, q] so the softmax sum is a
    matmul-reduction over partitions (ones column rides in the V tile).
  - Attention runs in causal q chunks with trimmed diagonal matmuls; the
    output projection and the second half of the V projection are spread
    through the score/PV loop as PE filler work so the exp throughput on
    the Activation engine never stalls the Tensor engine.
"""

import numpy as np
import ml_dtypes

import concourse.bass as bass
import concourse.tile as tile
from concourse import bacc, mybir
from concourse.bass_utils import run_bass_kernel_spmd

F32 = mybir.dt.float32
F32R = mybir.dt.float32r
BF16 = mybir.dt.bfloat16

B, S, E = 2, 2048, 1024
H, DH = 16, 64
NCORES = 8
HPC = 4          # heads per core
DL = HPC * DH    # 256: d_inner slice per core
NKT = E // 128   # 8  k-tiles over embed dim
NST = S // 128   # 16 seq tiles of 128

ExpF = mybir.ActivationFunctionType.Exp
IdF = mybir.ActivationFunctionType.Identity


def build_nc():
    nc = bacc.Bacc("TRN2", target_bir_lowering=False)

    xt_d = nc.dram_tensor("xt", [E, S], BF16, kind="ExternalInput")
    wq_d = nc.dram_tensor("wq", [E, DL], BF16, kind="ExternalInput")
    wk_d = nc.dram_tensor("wk", [E, DL], BF16, kind="ExternalInput")
    wv_d = nc.dram_tensor("wv", [E, DL], BF16, kind="ExternalInput")
    wo_d = nc.dram_tensor("wo", [DL, E], F32R, kind="ExternalInput")
    bqc_d = nc.dram_tensor("bqc", [DL, 1], F32, kind="ExternalInput")
    vone_d = nc.dram_tensor("v1ones", [128, 2 * HPC], BF16,
                            kind="ExternalInput")
    mask_d = nc.dram_tensor("masks", [128, 4, 1024], BF16,
                            kind="ExternalInput")
    out_d = nc.dram_tensor("out", [E, S], BF16, kind="ExternalOutput")

    with tile.TileContext(nc) as tc:
        with (
            tc.tile_pool(name="const", bufs=1) as cp,
            tc.tile_pool(name="ptp", bufs=16) as ptp,
        ):
            bqc = [cp.tile([128, 1], F32, tag=f"bqc{m}", name=f"bqc{m}")
                   for m in range(2)]

            qt = [cp.tile([128, S], F32R, tag=f"qt{m}", name=f"qt{m}")
                  for m in range(2)]
            kt = [cp.tile([128, S], F32R, tag=f"kt{m}", name=f"kt{m}")
                  for m in range(2)]
            ot = [cp.tile([128, S], F32R, tag=f"ot{m}", name=f"ot{m}")
                  for m in range(2)]
            v1 = [cp.tile([128, 2 * HPC * 65], BF16, tag=f"v1{s}",
                          name=f"v1{s}") for s in range(NST // 2)]
            wvt = cp.tile([128, NKT, DL], BF16, tag="wvt")
            wv = [wvt[:, k, :] for k in range(NKT)]
            wo = [cp.tile([128, E], F32R, tag=f"wo{d}", name=f"wo{d}")
                  for d in range(2)]
            maskt = cp.tile([128, 4, 1024], BF16, tag="maskt")
            masks = [maskt[:, j, :] for j in range(4)]
            wtqf = cp.tile([128, NKT, DL], BF16, tag="wtqf")
            wtkf = cp.tile([128, NKT, DL], BF16, tag="wtkf")
            wtq = [wtqf[:, :, m * 128:(m + 1) * 128] for m in range(2)]
            wtk = [wtkf[:, :, m * 128:(m + 1) * 128] for m in range(2)]
            # xt low columns (seq 0..1024) are only needed in phase 1; the
            # high half feeds the deferred V projection inside attention,
            # so it lives in the persistent pool.
            xtB = [cp.tile([128, S // 2], BF16, tag=f"xtB{k}",
                           name=f"xtB{k}") for k in range(NKT)]

            # pt (exp'd probability) tiles span both phases: early attention
            # blocks are score-computed inside phase 1 while the Activation
            # engine is otherwise idle.
            pend_map = {}   # (q0, hp) -> [(kb, pt, first, last, off), ...]

            def scores_kb(q0, qw, hp, i, kb, st_alloc):
                """One k block of scores for chunk block (q0, hp): matmul
                into a fresh stile, exp (+causal mask) into a pt tile, and
                append to the block's pending list."""
                nkb = (q0 + qw) // 128
                kb0 = q0 // 128
                j = kb - kb0
                off = (0 if qw != 512
                       else {1: 128, 2: 256, 3: 256}.get(j, 0))
                stile = st_alloc()
                ksl = slice(kb * 128, kb * 128 + 128)
                for h in range(2):
                    nc.tensor.matmul(
                        stile[:, h * qw + off:(h + 1) * qw],
                        kt[hp][h * 64:h * 64 + 64, ksl],
                        qt[hp][h * 64:h * 64 + 64, q0 + off:q0 + qw],
                        start=True, stop=True,
                    )
                pt = ptp.tile([128, 1024], BF16, tag="pt", name="pt")
                if off:
                    for h in range(2):
                        hs = slice(h * qw + off, (h + 1) * qw)
                        nc.scalar.activation(pt[:, hs], stile[:, hs],
                                             ExpF, scale=0.125)
                    with nc.allow_low_precision(
                            reason="0/1 mask multiply"):
                        for h in range(2):
                            hs = slice(h * qw + off, (h + 1) * qw)
                            nc.vector.tensor_mul(pt[:, hs], pt[:, hs],
                                                 masks[j][:, hs])
                else:
                    nc.scalar.activation(pt[:, 0:2 * qw], stile[:, 0:2 * qw],
                                         ExpF, scale=0.125)
                    if j >= 0:
                        with nc.allow_low_precision(
                                reason="0/1 mask multiply"):
                            nc.vector.tensor_mul(pt[:, 0:2 * qw],
                                                 pt[:, 0:2 * qw],
                                                 masks[j][:, 0:2 * qw])
                pend_map.setdefault((q0, hp), []).append(
                    (kb, pt, i == 0, i == nkb - 1, off))

            def diag_first(q0, qw):
                nkb = (q0 + qw) // 128
                kb0 = q0 // 128
                return list(range(kb0, nkb)) + list(range(0, kb0))

            # ============ phase 1 ============
            with (
                tc.tile_pool(name="xtp", bufs=1) as xp,
                tc.tile_pool(name="psqk", bufs=2, space="PSUM") as pqk,
            ):
                xtA = [xp.tile([128, S // 2], BF16, tag=f"xtA{k}",
                               name=f"xtA{k}") for k in range(NKT)]

                def xts(k, sb):
                    t = xtA[k] if sb < 2 else xtB[k]
                    return t[:, (sb % 2) * 512:(sb % 2) * 512 + 512]

                # DMA queue order = need order: first xt half-tile, k=0
                # weight slivers, rest of m=0 weights, small consts, the xt
                # stream, then V/m=1 weights, then attention-phase consts.
                nc.sync.dma_start(out=wtqf[:, 0, :], in_=wq_d[0:128, :])
                nc.sync.dma_start(out=xtA[0][:, 0:512],
                                  in_=xt_d[0:128, 0:512])
                nc.sync.dma_start(out=wtkf[:, 0, :], in_=wk_d[0:128, :])
                nc.sync.dma_start(out=xtA[0][:, 512:1024],
                                  in_=xt_d[0:128, 512:1024])
                nc.sync.dma_start(out=xtB[0][:], in_=xt_d[0:128, 1024:2048])
                nc.sync.dma_start(out=xtA[1][:],
                                  in_=xt_d[128:256, 0:1024])
                nc.sync.dma_start(
                    out=wtqf[:, 1:, :],
                    in_=wq_d[128:, :].rearrange("(k p) c -> p k c", p=128))
                nc.sync.dma_start(
                    out=wtkf[:, 1:, :],
                    in_=wk_d[128:, :].rearrange("(k p) c -> p k c", p=128))
                nc.sync.dma_start(out=xtB[1][:], in_=xt_d[128:256,
                                                          1024:2048])
                for mm in range(2):
                    nc.sync.dma_start(out=bqc[mm][:],
                                      in_=bqc_d[mm * 128:(mm + 1) * 128, :])
                for k in range(2, NKT):
                    nc.sync.dma_start(out=xtA[k][:],
                                      in_=xt_d[k * 128:(k + 1) * 128,
                                               0:1024])
                    nc.sync.dma_start(out=xtB[k][:],
                                      in_=xt_d[k * 128:(k + 1) * 128,
                                               1024:2048])
                nc.sync.dma_start(
                    out=wvt[:, 0:4, :],
                    in_=wv_d[0:512, :].rearrange("(k p) c -> p k c", p=128))
                nc.sync.dma_start(
                    out=wvt[:, 4:, :],
                    in_=wv_d[512:, :].rearrange("(k p) c -> p k c", p=128))
                nc.sync.dma_start(out=maskt[:], in_=mask_d[:])
                for s in range(NST // 2):
                    nc.sync.dma_start(
                        out=v1[s].rearrange("p (h c) -> p h c",
                                            c=65)[:, :, 64:65],
                        in_=vone_d.rearrange("p (h c) -> p h c", c=1)[:],
                    )
                for d in range(2):
                    nc.sync.dma_start(out=wo[d][:],
                                      in_=wo_d[d * 128:(d + 1) * 128, :])

                # -- Q and K projections, interleaved under the xt stream --
                # qt[m][d, s] = ((X @ Wq + bq).T)[m*128:(m+1)*128, :]
                def qk_proj(m):
                    pq = [pqk.tile([128, 1024], F32, tag="pq", name="pq")
                          for _ in range(2)]
                    pk = [pqk.tile([128, 1024], F32, tag="pk", name="pk")
                          for _ in range(2)]
                    for k in range(NKT):
                        for sb in range(4):
                            for (wt, ps) in ((wtq[m], pq), (wtk[m], pk)):
                                nc.tensor.matmul(
                                    ps[sb // 2][:,
                                                (sb % 2) * 512:
                                                (sb % 2) * 512 + 512],
                                    wt[:, k, :],
                                    xts(k, sb),
                                    start=(k == 0), stop=(k == NKT - 1),
                                )
                    # attention starts diagonal-first on the q block at
                    # 512:1024, so those kt/qt columns go out first (on
                    # separate engines); the ps[1] readers can be deferred
                    # so they don't delay releasing the ps[0] PSUM banks
                    nc.vector.tensor_copy(kt[m][:, 512:1024],
                                          pk[0][:, 512:1024])
                    nc.scalar.activation(qt[m][:, 512:1024],
                                         pq[0][:, 512:1024],
                                         IdF, bias=bqc[m][:])
                    nc.vector.tensor_copy(kt[m][:, 0:512], pk[0][:, 0:512])
                    nc.scalar.activation(qt[m][:, 0:512], pq[0][:, 0:512],
                                         IdF, bias=bqc[m][:])

                    def ps1_epilogue():
                        with nc.allow_low_precision(
                                reason="f32r round of q + bias"):
                            nc.vector.tensor_scalar_add(
                                qt[m][:, 1024:2048], pq[1][:], bqc[m][:])
                        nc.scalar.activation(kt[m][:, 1024:2048],
                                             pk[1][:], IdF)
                    return ps1_epilogue

                qk0_ps1 = qk_proj(0)

                # -- V projection, first half (activations stationary) --
                # v1[st][s, 65h:65h+64] = (X @ Wv)[st*128.., 64h:64h+64];
                # second half (st 8..15) is emitted later as PE filler work
                # inside the attention phase, since causal PV only touches
                # high k blocks late.  The hp=0 score+exp work of the first
                # two attention chunks rides along here: the Activation
                # engine is idle in phase 1, and pulling those exps forward
                # keeps it off the critical path later.
                st_eng = [0]

                def st_alloc():
                    st_eng[0] ^= 1
                    return pqk.tile([128, 1024], F32,
                                    tag=("pq" if st_eng[0] else "pk"),
                                    name="sa_stile")

                ahead = [(512, 512, 0, i, kb)
                         for i, kb in enumerate(diag_first(512, 512))]
                ahead += [(0, 512, 0, i, kb)
                          for i, kb in enumerate(diag_first(0, 512))]
                for sp2 in range(NST // 4):
                    pv = pqk.tile([128, 512], F32,
                                  tag=("pq" if sp2 % 2 else "pk"), name="pv")
                    for i in range(2):
                        st = 2 * sp2 + i
                        for k in range(NKT):
                            nc.tensor.matmul(
                                pv[:, i * DL:(i + 1) * DL],
                                xtA[k][:, st * 128:(st + 1) * 128],
                                wv[k],
                                start=(k == 0), stop=(k == NKT - 1),
                            )
                    vv = v1[sp2].rearrange("p (h c) -> p h c", c=65)
                    pvv = pv[:].rearrange("p (h c) -> p h c", c=64)
                    if sp2 % 2:
                        nc.scalar.copy(out=vv[:, :, 0:64], in_=pvv[:])
                    else:
                        nc.vector.tensor_copy(vv[:, :, 0:64], pvv[:])
                    for _ in range(3):
                        if ahead:
                            scores_kb(*ahead.pop(0), st_alloc)
                    if sp2 == 0:
                        qk0_ps1()
                while ahead:
                    scores_kb(*ahead.pop(0), st_alloc)

                qk_proj(1)()

            # ============ phase 2 (xtA region reused) ============
            with (
                tc.tile_pool(name="small", bufs=2) as sp,
                tc.tile_pool(name="oev", bufs=2) as op,
                tc.tile_pool(name="psatt", bufs=1, space="PSUM") as pat,
            ):
                fillers = []

                def pop_filler(reserve=0):
                    if len(fillers) > reserve:
                        fillers.pop(0)()

                copy_eng = [0]

                # one output-staging buffer per chunk: 8 et copies land
                # here, then a single batched DMA writes the whole chunk
                oeb = {}

                def oproj(q0, qw, et):
                    p3 = pat.tile([128, 512], F32, tag="pp", bufs=2,
                                  name="p3")
                    esl = slice(et * 128, et * 128 + 128)
                    for d in range(2):
                        nc.tensor.matmul(
                            p3[:, 0:qw],
                            wo[d][:, esl],
                            ot[d][:, q0:q0 + qw],
                            start=(d == 0), stop=(d == 1),
                        )
                    if q0 not in oeb:
                        oeb[q0] = op.tile([128, NKT, 512], BF16, tag="oe",
                                          name="oe")
                    oe = oeb[q0]
                    copy_eng[0] ^= 1
                    if copy_eng[0]:
                        nc.scalar.copy(out=oe[:, et, 0:qw], in_=p3[:, 0:qw])
                    else:
                        nc.vector.tensor_copy(oe[:, et, 0:qw], p3[:, 0:qw])
                    if et % 2 == 1:
                        nc.sync.dma_start(
                            out=out_d[(et - 1) * 128:(et + 1) * 128,
                                      q0:q0 + qw].rearrange(
                                "(e p) q -> p e q", p=128),
                            in_=oe[:, et - 1:et + 1, 0:qw],
                        )
                        if et == NKT - 1:
                            del oeb[q0]

                def vproj_st(st):
                    # one 128-seq tile of the deferred V projection
                    pv = pat.tile([128, 512], F32, tag="pp", bufs=2,
                                  name="pv")
                    for k in range(NKT):
                        nc.tensor.matmul(
                            pv[:, 0:DL],
                            xtB[k][:, (st - 8) * 128:(st - 7) * 128],
                            wv[k],
                            start=(k == 0), stop=(k == NKT - 1),
                        )
                    g = (st % 2) * 260
                    vv = v1[st // 2][:, g:g + 260].rearrange(
                        "p (h c) -> p h c", c=65)
                    pvv = pv[:, 0:DL].rearrange("p (h c) -> p h c", c=64)
                    nc.vector.tensor_copy(vv[:, :, 0:64], pvv[:])

                # ---- globally pipelined attention ----
                # Blocks run one behind: block n's PV drain and softmax
                # normalize interleave through block n+1's score stream, so
                # reciprocal/broadcast latency hides behind stiles and the
                # Act-vs-PE rate gap is bridged by oproj/V filler pops.
                chunks = [(512, 512), (0, 512), (1024, 512), (1536, 512)]
                fillers += [(lambda st=st: vproj_st(st))
                            for st in range(8, 16)]

                def st_alloc():
                    return pat.tile([128, 1024], F32, tag="st",
                                    bufs=2, name="stile")

                po_of = {}    # block idx -> (po_a, po_b)
                live = []     # FIFO of (block idx, pend entry)
                blocks = [dict(q0=q0, qw=qw, hp=hp)
                          for (q0, qw) in chunks for hp in range(2)]

                def normalize(b):
                    # recip denominators land on partition 0 (vector ops may
                    # shift partition offsets), partition_broadcast fans them
                    # out (it can only read partition 0), and the multiplies
                    # write both ot halves in place (shifting up for h=1).
                    q0, qw, hp = b["q0"], b["qw"], b["hp"]
                    po_a, po_b = po_of.pop(b["idx"])
                    rb = sp.tile([1, 1024], F32R, tag="rb", name="rb")
                    with nc.allow_low_precision(
                            reason="f32r rounding of softmax denoms"):
                        nc.vector.reciprocal(rb[0:1, 0:qw],
                                             po_b[64:65, 0:qw])
                        nc.vector.reciprocal(rb[0:1, 512:512 + qw],
                                             po_a[64:65, 0:qw])
                    for h, po in ((1, po_b), (0, po_a)):
                        c0 = 0 if h else 512
                        rbb = sp.tile([64, 512], F32R, tag="rbb", name="rbb")
                        nc.gpsimd.partition_broadcast(
                            rbb[:, 0:qw], rb[0:1, c0:c0 + qw])
                        nc.vector.tensor_mul(
                            ot[hp][h * 64:h * 64 + 64, q0:q0 + qw],
                            po[0:64, 0:qw], rbb[:, 0:qw])
                    if hp == 1:
                        fillers.extend(
                            [(lambda q0=q0, qw=qw, et=et: oproj(q0, qw, et))
                             for et in range(NKT)])

                def pv_pop():
                    bi, pend = live.pop(0)
                    b = blocks[bi]
                    qw, hp = b["qw"], b["hp"]
                    if bi not in po_of:
                        po_of[bi] = (
                            pat.tile([128, 512], F32, tag="po", bufs=2,
                                     name="po_a"),
                            pat.tile([128, 512], F32, tag="po", bufs=2,
                                     name="po_b"))
                    po_a, po_b = po_of[bi]
                    pkb, ppt, pfirst, plast, poff = pend
                    base = (pkb % 2) * 260
                    for h, po in ((0, po_a), (1, po_b)):
                        lh = 2 * hp + h
                        nc.tensor.matmul(
                            po[0:65, poff:qw],
                            v1[pkb // 2][:, base + lh * 65:
                                         base + lh * 65 + 65],
                            ppt[:, h * qw + poff:(h + 1) * qw],
                            start=pfirst, stop=plast,
                            skip_group_check=True,
                        )
                    if plast:
                        normalize(b)

                nsc = [0]
                krsv = [4]

                def after_score():
                    nsc[0] += 1
                    while len(live) >= 12:
                        pv_pop()
                    if nsc[0] % 3 == 0:
                        pop_filler(krsv[0])

                for bi, b in enumerate(blocks):
                    b["idx"] = bi
                    # hold fillers back mid-schedule; spend them at the
                    # final drain where the score stream has ended
                    krsv[0] = 6 if bi == len(blocks) - 1 else (
                        0 if b["q0"] == 0 else (4 if nsc[0] <= 40 else 0))
                    q0, qw, hp = b["q0"], b["qw"], b["hp"]
                    pre = pend_map.pop((q0, hp), [])
                    npre = len(pre)
                    live.extend((bi, p) for p in pre)
                    nkb = (q0 + qw) // 128
                    if npre >= nkb:
                        # fully preloaded: drain burst keeps PE fed while
                        # the Activation engine idles here
                        while len(live) >= 12:
                            pv_pop()
                        continue
                    # drain anything two blocks back before a new po pair
                    # could be needed mid-stream
                    while live and live[0][0] <= bi - 2:
                        pv_pop()
                    for i, kb in enumerate(diag_first(q0, qw)):
                        if i < npre:
                            continue
                        scores_kb(q0, qw, hp, i, kb, st_alloc)
                        live.append((bi, pend_map[(q0, hp)].pop(0)))
                        after_score()
                    pend_map.pop((q0, hp), None)
                npv = [0]
                while live:
                    pv_pop()
                    npv[0] += 1
                    if npv[0] % 1 == 0:
                        pop_filler()
                while fillers:
                    pop_filler()

    nc.compile()
    return nc


def _make_masks(qw, nj):
    kk = np.arange(128)[:, None]
    qq = np.arange(qw)[None, :]
    ms = []
    for j in range(nj):
        m = np.where(qq >= kk + 128 * j, 1.0, 0.0).astype(np.float32)
        ms.append(np.concatenate([m, m], axis=1))
    return np.ascontiguousarray(np.stack(ms).transpose(1, 0, 2))


_NC = None


def _get_nc():
    global _NC
    if _NC is None:
        _NC = build_nc()
    return _NC


def make_in_maps(inputs, Wq, bq, Wk, Wv, Wo):
    bf = ml_dtypes.bfloat16
    masks = _make_masks(512, 4).astype(bf)
    vones = np.ones((128, 2 * HPC), bf)
    in_maps = []
    for c in range(NCORES):
        b, g = c // HPC, c % HPC
        sl = slice(g * DL, (g + 1) * DL)
        in_maps.append({
            "xt": np.ascontiguousarray(np.asarray(inputs[b]).T.astype(bf)),
            "wq": np.ascontiguousarray(Wq[:, sl].astype(bf)),
            "wk": np.ascontiguousarray(Wk[:, sl].astype(bf)),
            "wv": np.ascontiguousarray(Wv[:, sl].astype(bf)),
            "wo": np.ascontiguousarray(Wo[sl, :]),
            "bqc": bq[sl].reshape(DL, 1),
            "v1ones": vones,
            "masks": masks,
        })
    return in_maps


def assemble(results):
    outs = []
    for b in range(B):
        acc = results[b * HPC]["out"].astype(np.float32).copy()
        for g in range(1, HPC):
            acc += results[b * HPC + g]["out"]
        outs.append(acc.T)
    return np.stack(outs)


def kernel(inputs, Wq, bq, Wk, bk, Wv, bv, Wo, bo):
    inputs = np.asarray(inputs, np.float32)
    Wq, bq, Wk, Wv, Wo = (np.asarray(a, np.float32)
                          for a in (Wq, bq, Wk, Wv, Wo))
    bv = np.asarray(bv, np.float32)
    bo_eff = np.asarray(bo, np.float32) + bv @ np.asarray(Wo, np.float32)
    in_maps = make_in_maps(inputs, Wq, bq, Wk, Wv, Wo)
    nc = _get_nc()
    res = run_bass_kernel_spmd(nc, in_maps, list(range(NCORES)))
    out = assemble(res.results)
    return (out + bo_eff).astype(np.float32)
